# revision 28
# baseline (speedup 1.0000x reference)
"""Blockwise K/V selector (sparse attention) on 8 Trainium2 NeuronCores.

Full computation on device:
  scores = q . compressed_keys / sqrt(D)  -> softmax -> GQA mean-pool over
  heads -> top-16 blocks (sign-rank trick, no sort) -> one fused
  indirect-DMA gather of the selected K+V 64-row blocks per (b, g) pair.

Sharding: the 16 (b, g) pairs are fully independent; each of the 8 cores
processes 2 pairs (pure data parallel, no collectives).

Data movement (memory roofline is the target, ~3.1 MB/core/iter):
  * q and ck are uploaded PRE-TRANSPOSED ([D, heads] / [D, (pair head n)])
    so the score matmuls need no on-device PE transposes or PSUM copies.
    Scoring stays f32: the pooled-prob gap at the rank-16 boundary is as
    small as 1e-6 on this input, so bf16/fp16 scores would flip blocks.
  * K and V are uploaded as ONE bf16 tensor [PAIRS, 2, S, D]; the gather
    output is written bf16 and up-cast to f32 on the host. bf16 is a pure
    0.4%-max quantization of the gathered values (far below the 2e-2
    tolerance) and halves both gather and store HBM traffic.
  * Per pair a single 128-index indirect DMA (16 rows = 4 KiB bf16 per
    index, the max span one dest partition line supports) gathers K and V
    together; one HWDGE store per pair (SP ring / ACT ring).

Engine budget (each DVE op costs its duration AGAIN in pipeline DRAIN, so
DVE is cut to 5 tiny ops; exp/copy/sign all live in the one ACT table
"exp_and_others" so ACT never reloads tables; Pool runs only the two
gather descriptor-gens so they are never queued behind masks):
  * M[p, c] = A[c] - A[p] built in PSUM by two accumulating matmuls whose
    MAC sequences are term-wise identical up to exact negation (weights
    rz broadcast vs e with -rz broadcast), so the diagonal is EXACTLY 0.
  * rank via s[c] = sum_p sign(M[p, c]) = 127 - 2*rank[c]: Sign on ACT
    (reads PSUM), one PE matmul against a bf16 ones column. Verified on
    this input: the min pairwise pooled-prob gap is 82 ulps, so exact
    f32 ties cannot occur and sign() needs no tie/diagonal masks.
  * selection compares against the REMAPPED constant 127 - 2*slot(c), so
    no rank decode is needed; chunk bases come from one matmul vs 4p.
"""
import os
import numpy as np

B = 4
H = 32
G = 4
HPG = H // G          # 8 heads per query group
PAIRS = 2             # (b, g) pairs per core
N = 128               # number of compressed keys / key blocks
D = 128               # head dim
S = 8192              # kv sequence length
BS = 64               # block size
NSEL = 16             # selected blocks
NCORES = 8
# gather granularity: 16 bf16 rows = 4 KiB per index. The indirect-DMA DGE
# maps one index to one dest SBUF partition, so the per-index span must equal
# one partition line of the dest tile (4 KiB) — larger spans corrupt on HW.
CHUNK = 16
RPB = BS // CHUNK     # chunks per block (4)
NCHUNK = 2 * NSEL * RPB  # 128 chunks per pair: 64 K-chunks then 64 V-chunks
SCALE = 1.0 / float(D) ** 0.5
GH = PAIRS * HPG      # 16 heads handled per core

_CACHE = {}
LAST_RESULT = None    # BassKernelResults of the most recent run (for test.py)


def _build_nc():
    import concourse.bass as bass
    import concourse.bacc as bacc
    import concourse.mybir as mybir
    import concourse.tile as tile

    F32 = mybir.dt.float32
    BF16 = mybir.dt.bfloat16

    nc = bacc.Bacc("TRN2", target_bir_lowering=False, debug=False)

    qt_in = nc.dram_tensor("qt_in", [D, GH], F32, kind="ExternalInput")
    ckt_in = nc.dram_tensor("ckt_in", [D, GH * N], F32, kind="ExternalInput")
    kv_in = nc.dram_tensor("kv_in", [PAIRS, 2, S, D], BF16, kind="ExternalInput")
    # bf16 consts: iotabh2 = 127-2*slot(c) (128) | pvec 4p | ones |
    # signmat sign(p-c) (128)
    cb_in = nc.dram_tensor("cb_in", [128, 258], BF16, kind="ExternalInput")
    # f32 consts: identity (128) | cvec per pair (2) | +1 | -1 |
    # negated identity (128)
    cf_in = nc.dram_tensor("cf_in", [128, 260], F32, kind="ExternalInput")
    out_kv = nc.dram_tensor("out_kv", [PAIRS, 2, NSEL * BS, D], BF16,
                            kind="ExternalOutput")
    dbg = None
    if int(os.environ.get("KDEBUG", "0")):
        dbg = nc.dram_tensor("dbg", [128, 8], mybir.dt.float32,
                             kind="ExternalOutput")

    # flat chunk view for the gather: [(p t c) = 2048 chunks, 2048 elems]
    kv_flat = kv_in[:].rearrange("p t (c r) d -> (p t c) (r d)", r=CHUNK)

    # KREPEAT>1 builds the pipeline several times (serialized by the
    # TileContext exit barrier) so device time can be measured as the
    # marginal wall-clock per repeat. KEMPTY=1 emits no-op contexts for
    # calibrating the barrier cost.
    repeat = int(os.environ.get("KREPEAT", "1"))
    empty = bool(int(os.environ.get("KEMPTY", "0")))
    # KSTAGE (timing ablation only): 1=loads, 5=+scores/exp, 6=+M, 2=all
    # compute, 4=loads+const-idx gathers+stores, 0=full
    stage = int(os.environ.get("KSTAGE", "0"))
    for _rep in range(repeat):
        _emit_once(nc, tc_mod=tile, bassmod=bass, mybirmod=mybir, empty=empty,
                   stage=stage,
                   tensors=(qt_in, ckt_in, kv_flat, cb_in, cf_in, out_kv, dbg))

    nc.compile()
    return nc


def _emit_once(nc, tc_mod, bassmod, mybirmod, empty, tensors, stage=0):
    bass = bassmod
    mybir = mybirmod
    tile = tc_mod
    (qt_in, ckt_in, kv_flat, cb_in, cf_in, out_kv, dbg) = tensors
    F32 = mybir.dt.float32
    BF16 = mybir.dt.bfloat16
    I32 = mybir.dt.int32
    Alu = mybir.AluOpType
    Act = mybir.ActivationFunctionType
    Ax = mybir.AxisListType

    with tile.TileContext(nc) as tc:
        if empty:
            with tc.tile_pool(name="noop", bufs=1) as np_:
                t = np_.tile([1, 1], F32)
                nc.vector.memset(t[:], 0.0)
            return
        with tc.tile_pool(name="consts", bufs=1) as cp, \
             tc.tile_pool(name="work", bufs=1) as wp, \
             tc.tile_pool(name="psum", bufs=1, space="PSUM") as pp:

            # ---- loads: ckt halves on SP ring, q + consts on ACT ring ----
            ckt_sb = cp.tile([D, GH * N], F32)
            for p in range(PAIRS):
                nc.sync.dma_start(
                    out=ckt_sb[:, p * HPG * N:(p + 1) * HPG * N],
                    in_=ckt_in[:, p * HPG * N:(p + 1) * HPG * N])
            qt_sb = cp.tile([D, GH], F32)
            nc.scalar.dma_start(out=qt_sb[:], in_=qt_in[:])
            cb = cp.tile([128, 258], BF16)
            nc.scalar.dma_start(out=cb[:], in_=cb_in[:])
            cf = cp.tile([128, 260], F32)
            nc.scalar.dma_start(out=cf[:], in_=cf_in[:])
            iotabh2 = cb[:, 0:128]
            pvec = cb[:, 128:129]
            onesb = cb[:, 129:130]
            signmat = cb[:, 130:258]
            ident = cf[:, 0:128]
            cvec = cf[:, 128:130]
            onesc = cf[:, 130:131]
            monesc = cf[:, 131:132]
            nident = cf[:, 132:260]

            if stage == 1:
                return
            if stage == 4:
                # timing probe: gathers+stores with constant indices
                for p in range(PAIRS):
                    idxc = wp.tile([128, 1], I32, tag=f"idxc{p}")
                    nc.gpsimd.iota(idxc[:], pattern=[[0, 1]], base=p * 1024,
                                   channel_multiplier=1)
                    kvsel = wp.tile([128, NCHUNK * CHUNK * D // 128], BF16,
                                    tag=f"kvsel{p}")
                    nc.gpsimd.indirect_dma_start(
                        out=kvsel[:], out_offset=None, in_=kv_flat,
                        in_offset=bass.IndirectOffsetOnAxis(ap=idxc[:, :1],
                                                            axis=0))
                    eng = nc.sync if p == 0 else nc.scalar
                    eng.dma_start(
                        out=out_kv[p].rearrange("t (s j r) d -> (t s j) (r d)",
                                                j=RPB, r=CHUNK),
                        in_=kvsel[:])
                return

            # ---- PE: scores; per-pair z (e @ +-1) and e^T ----
            sc_ps = pp.tile([N, GH], F32, tag="sc")
            zz_ps = pp.tile([HPG, 2 * PAIRS], F32, tag="zz")
            eT_ps = pp.tile([HPG, PAIRS * N], F32, tag="et")
            esb = wp.tile([N, GH], F32)
            for p in range(PAIRS):
                hs = slice(p * HPG, (p + 1) * HPG)
                for h in range(HPG):
                    g = p * HPG + h
                    nc.tensor.matmul(
                        out=sc_ps[:, g:g + 1],
                        lhsT=ckt_sb[:, g * N:(g + 1) * N],
                        rhs=qt_sb[:, g:g + 1],
                        start=True, stop=True)
                # ACT: softmax numerator, no max-subtraction (scores ~
                # N(0,1) after scaling; matches jax to ~1e-7 relative,
                # far below the top-k prob gaps)
                nc.scalar.activation(out=esb[:, hs], in_=sc_ps[:, hs],
                                     func=Act.Exp, scale=SCALE)
                # z and -z columns for this pair
                nc.tensor.matmul(out=zz_ps[:, 2 * p:2 * p + 1],
                                 lhsT=esb[:, hs], rhs=onesc[:],
                                 start=True, stop=True)
                nc.tensor.matmul(out=zz_ps[:, 2 * p + 1:2 * p + 2],
                                 lhsT=esb[:, hs], rhs=monesc[:],
                                 start=True, stop=True)
                nc.tensor.transpose(out=eT_ps[:, p * N:(p + 1) * N],
                                    in_=esb[:, hs], identity=ident[:])

            if stage == 5:
                return

            # ---- DVE: rz = 1/z and -rz = 1/(-z) per pair (tiny);
            # ACT: e^T PSUM -> SBUF copies ----
            rzz, e_sbl = [], []
            for p in range(PAIRS):
                r = wp.tile([HPG, 2], F32, tag=f"rz{p}")
                nc.vector.reciprocal(out=r[:], in_=zz_ps[:, 2 * p:2 * p + 2])
                es = wp.tile([HPG, N], F32, tag=f"es{p}")
                nc.scalar.copy(out=es[:], in_=eT_ps[:, p * N:(p + 1) * N])
                rzz.append(r)
                e_sbl.append(es)

            # ---- M[p, c] = A[c] - A[p] per pair, built in PSUM by two
            # accumulating matmuls whose MAC sequences are term-wise
            # identical up to exact negation -> diagonal exactly 0 ----
            m_ps = []
            for p in range(PAIRS):
                m = pp.tile([128, 128], F32, tag=f"m{p}")
                nc.tensor.matmul(out=m[:],
                                 lhsT=rzz[p][:, 0:1].to_broadcast([HPG, N]),
                                 rhs=e_sbl[p][:], start=True, stop=False)
                nc.tensor.matmul(out=m[:], lhsT=e_sbl[p][:],
                                 rhs=rzz[p][:, 1:2].to_broadcast([HPG, N]),
                                 start=False, stop=False)
                # M -= I: the two A matmuls quantize fp32 differently on the
                # PE (weights vs moving operand), so the diagonal is ~2e-9,
                # not 0. 1.0*(-1.0) is exact in any decomposition, so this
                # pins the diagonal strictly negative -> sign(diag) = -1
                # always, absorbed into the 126-2*slot sel constant.
                nc.tensor.matmul(out=m[:], lhsT=ident[:], rhs=nident[:],
                                 start=False, stop=True)
                m_ps.append(m)
            if stage == 6:
                dump = wp.tile([128, 2], F32, tag="dump")
                for p in range(PAIRS):
                    nc.vector.tensor_reduce(out=dump[:, p:p + 1],
                                            in_=m_ps[p][:], op=Alu.add,
                                            axis=Ax.X)
                return

            # ---- ACT: sg = sign(M); DVE: eq = [M == 0] (PE fp32r rounding
            # makes bitwise A-collisions real — observed on HW — so exact
            # ties DO occur and carry sign 0); Pool: Z = eq * sign(p-c)
            # resolves each tie by index (lower index wins, matching
            # jax.top_k); PE: s[c] = sum_p (sg + Z)[p, c] = 127-2*rank[c] ----
            s_ps = pp.tile([128, PAIRS], F32, tag="s")
            sgl, zl = [], []
            for p in range(PAIRS):
                sg = wp.tile([128, 128], BF16, tag=f"sg{p}")
                # scale pushes tiny-but-nonzero M out of the sign table's
                # zero bin; exact zeros stay zero (repaired via Z below)
                nc.scalar.activation(out=sg[:], in_=m_ps[p][:], func=Act.Sign,
                                     scale=1e30)
                eq = wp.tile([128, 128], BF16, tag=f"eq{p}")
                nc.vector.tensor_scalar(
                    out=eq[:], in0=m_ps[p][:], scalar1=0.0, scalar2=None,
                    op0=Alu.is_equal)
                z_ = wp.tile([128, 128], BF16, tag=f"z{p}")
                nc.gpsimd.tensor_tensor(
                    out=z_[:], in0=eq[:], in1=signmat[:], op=Alu.mult)
                sgl.append(sg)
                zl.append(z_)
            for p in range(PAIRS):
                nc.tensor.matmul(out=s_ps[:, p:p + 1], lhsT=sgl[p][:],
                                 rhs=onesb[:], start=True, stop=False)
                nc.tensor.matmul(out=s_ps[:, p:p + 1], lhsT=zl[p][:],
                                 rhs=onesb[:], start=False, stop=True)

            # ---- DVE: selection matrix vs remapped slot consts;
            # PE: chunk bases; DVE: final int32 indices ----
            ch_ps = pp.tile([128, PAIRS], F32, tag="ch")
            idxil = []
            for p in range(PAIRS):
                sel = wp.tile([128, NCHUNK], BF16, tag=f"sel{p}")
                nc.vector.tensor_scalar(
                    out=sel[:], in0=iotabh2[:], scalar1=s_ps[:, p:p + 1],
                    scalar2=None, op0=Alu.is_equal)
                nc.tensor.matmul(out=ch_ps[:, p:p + 1], lhsT=sel[:],
                                 rhs=pvec[:], start=True, stop=True)
                idxi = wp.tile([128, 1], I32, tag=f"idxi{p}")
                nc.vector.tensor_tensor(
                    out=idxi[:], in0=ch_ps[:, p:p + 1],
                    in1=cvec[:, p:p + 1], op=Alu.add)
                idxil.append(idxi)

            if dbg is not None:
                dw = wp.tile([128, 8], F32, tag="dw")
                nc.vector.tensor_copy(out=dw[:, 0:2], in_=s_ps[:])
                nc.vector.tensor_copy(out=dw[:, 2:3], in_=m_ps[0][:, 1:2])
                nc.vector.tensor_copy(out=dw[:, 3:4], in_=sgl[0][:, 1:2])
                nc.vector.tensor_copy(out=dw[:, 4:5], in_=m_ps[0][:, 0:1])
                nc.vector.tensor_copy(out=dw[:, 5:6], in_=sgl[0][:, 0:1])
                nc.vector.tensor_copy(out=dw[:, 6:7], in_=zl[0][:, 1:2])
                nc.vector.tensor_copy(out=dw[:, 7:8], in_=zl[0][:, 0:1])
                nc.sync.dma_start(out=dbg[:], in_=dw[:])
            if stage == 2:
                return
            # ---- fused K+V gather (128 chunks x 4 KiB each) and store;
            # p0 store on SP ring, p1 store on ACT ring ----
            for p in range(PAIRS):
                kvsel = wp.tile([128, NCHUNK * CHUNK * D // 128], BF16,
                                tag=f"kvsel{p}")
                nc.gpsimd.indirect_dma_start(
                    out=kvsel[:], out_offset=None, in_=kv_flat,
                    in_offset=bass.IndirectOffsetOnAxis(ap=idxil[p][:, :1],
                                                        axis=0))
                if stage == 3:
                    continue
                eng = nc.sync if p == 0 else nc.scalar
                eng.dma_start(
                    out=out_kv[p].rearrange("t (s j r) d -> (t s j) (r d)",
                                            j=RPB, r=CHUNK),
                    in_=kvsel[:])


def _consts():
    import ml_dtypes
    cb = np.zeros((128, 258), dtype=np.float32)
    c = np.arange(NCHUNK, dtype=np.float32)
    cb[:, 0:128] = (126.0 - 2.0 * ((c % (NSEL * RPB)) // RPB))[None, :]
    cb[:, 128] = float(RPB) * np.arange(128, dtype=np.float32)
    cb[:, 129] = 1.0
    pi = np.arange(128, dtype=np.float32)
    cb[:, 130:258] = np.sign(pi[:, None] - pi[None, :])
    cf = np.zeros((128, 260), dtype=np.float32)
    cf[:, 0:128] = np.eye(128, dtype=np.float32)
    # cvec[c, p] = p * (2*S//CHUNK) + (c // 64) * (S//CHUNK) + c % RPB
    ci = np.arange(128, dtype=np.float32)
    cf[:, 128:130] = (np.arange(PAIRS, dtype=np.float32)[None, :]
                      * (2 * S // CHUNK)
                      + (ci[:, None] // (NSEL * RPB)) * (S // CHUNK)
                      + (ci[:, None] % RPB))
    cf[:, 130] = 1.0
    cf[:, 131] = -1.0
    cf[:, 132:260] = -np.eye(128, dtype=np.float32)
    return {"cb_in": cb.astype(ml_dtypes.bfloat16),
            "cf_in": np.ascontiguousarray(cf)}


def _in_maps_from_full(query, compressed_keys, keys, values):
    """Shard + pre-transpose the full inputs into per-core in_maps."""
    import ml_dtypes
    consts = _consts()
    in_maps = []
    for core in range(NCORES):
        bs, gs = [], []
        for j in range(PAIRS):
            f = PAIRS * core + j
            bs.append(f // G)
            gs.append(f % G)
        # qt [D, GH]: column p*HPG+h = q[b_p, g_p*HPG+h, -1, :]
        q_s = np.stack([query[b, g * HPG:(g + 1) * HPG, -1, :]
                        for b, g in zip(bs, gs)])          # [P, HPG, D]
        qt = np.ascontiguousarray(q_s.reshape(GH, D).T)     # [D, GH]
        # ckt [D, GH*N]
        ck_s = np.stack([compressed_keys[b, g * HPG:(g + 1) * HPG]
                         for b, g in zip(bs, gs)])          # [P, HPG, N, D]
        ckt = np.ascontiguousarray(
            ck_s.reshape(GH * N, D).T)                      # [D, GH*N]
        # kv bf16 [P, 2, S, D]
        kv = np.stack([np.stack([keys[b, g], values[b, g]])
                       for b, g in zip(bs, gs)])
        kv = kv.astype(ml_dtypes.bfloat16)
        im = {"qt_in": qt, "ckt_in": ckt, "kv_in": np.ascontiguousarray(kv)}
        im.update(consts)
        in_maps.append(im)
    return in_maps


def kernel(query, compressed_keys, keys, values):
    global LAST_RESULT
    from concourse.bass_utils import run_bass_kernel_spmd

    query = np.asarray(query, dtype=np.float32)
    compressed_keys = np.asarray(compressed_keys, dtype=np.float32)
    keys = np.asarray(keys, dtype=np.float32)
    values = np.asarray(values, dtype=np.float32)

    key = (os.environ.get("KREPEAT", "1"), os.environ.get("KEMPTY", "0"),
           os.environ.get("KSTAGE", "0"))
    if key not in _CACHE:
        _CACHE[key] = _build_nc()
    nc = _CACHE[key]

    in_maps = _in_maps_from_full(query, compressed_keys, keys, values)
    res = run_bass_kernel_spmd(nc, in_maps, list(range(NCORES)))
    LAST_RESULT = res

    sel_k = np.empty((B, G, NSEL * BS, D), dtype=np.float32)
    sel_v = np.empty((B, G, NSEL * BS, D), dtype=np.float32)
    for core in range(NCORES):
        for j in range(PAIRS):
            f = PAIRS * core + j
            b, g = f // G, f % G
            okv = np.asarray(res.results[core]["out_kv"][j])
            sel_k[b, g] = okv[0].astype(np.float32)
            sel_v[b, g] = okv[1].astype(np.float32)
    return sel_k, sel_v
